# revision 12
# baseline (speedup 1.0000x reference)
"""Trainium2 Bass kernel for the CustomRNN problem (v5).

Model (per batch element b):
    u_t = W_in @ x_t + bias + sigma*sqrt(2*alpha) * noise_t          [N=256]
    r_{t+1} = (1-alpha) * r_t + alpha * relu(W_rec @ r_t + u_t)
    out_t = W_out @ r_{t+1} + b_out                                  [3]

Sharding: data-parallel over batch across 8 cores (32 batch each), weights
replicated.

v5 design notes: the run is latency-bound by the per-step serial cycle
    STT(DVE) -> sem -> 4 chain matmuls -> PSUM pipe -> sem -> STT'
with G=2 staggered 16-batch chains.  In v2..v4 the two groups' state
updates wrote ONE hist tile, so each group's chain matmuls waited on BOTH
groups' DVE ops (coarse region dep + counting semaphore) — putting
STT_A+STT_B (~240ns) on the critical path.  v5 gives each group its OWN
hist tile, so group A's cycle only contains its own STT: predicted cycle
~470ns instead of ~553ns.
Filler work (drive/noise prefill for the next block, y projection of the
previous block) is emitted at the top of designated steps, one filler
TYPE per slot (mixed-stationary filler runs pay ~150ns boundary holes),
each piece <= 128 cols so it never monopolizes the array (the in-order PE
queue turns long fillers into chain stalls).  Keep total PE array duty
comparable to v2 — dropping it demotes the PE p-state and slows the chain
(observed v3: MID->LOW pstate, everything x1.5).
Numerics identical to v2 (fp16 recurrence with exact fp16-decay rescaling,
fp8 x16 noise via identity matmuls, fp32 PSUM).
"""

import numpy as np

import concourse.bacc as bacc
import concourse.mybir as mybir
from concourse.tile import TileContext, add_dep_helper
from concourse.bass_utils import run_bass_kernel_spmd

ALPHA = 0.2
NOISE_SCALE = 0.05 * float(np.sqrt(2 * ALPHA))
DECAY = float(np.float16(1.0 - ALPHA))   # 0.7998046875, exact in fp16
N = 256
NCORES = 8
BC = 32          # batch per core
F16 = mybir.dt.float16
F32 = mybir.dt.float32
F8 = mybir.dt.float8e4      # e4m3
F8NP = mybir.dt.np(mybir.dt.float8e4)
NOISE_PREMUL = 16.0         # fp8 noise stored x16; identity diag = 1/16

_CACHE = {}


def _chunks(T, TC, first):
    """Chunk sizes: a small first chunk (cold-start DMA off the critical
    path), then TC-sized chunks, remainder absorbed at the end."""
    out = []
    o = 0
    if first and T > first:
        out.append((0, first))
        o = first
    while o < T:
        n = min(TC, T - o)
        out.append((o, n))
        o += n
    assert all(n % 8 == 0 for _, n in out)
    return out


def _build(T, TC, SB, G, first=16):
    GB = BC // G
    assert G * GB == BC and SB * 2 * GB * 4 <= 2048 and TC % SB == 0
    CHUNKS = _chunks(T, TC, first)
    nc = bacc.Bacc("TRN2", num_devices=NCORES)

    noise_d = nc.dram_tensor("noiset", [128, T, 2 * BC], F8, kind="ExternalInput")
    xta_d = nc.dram_tensor("xta", [4, T, BC], F16, kind="ExternalInput")
    # all fp16 constants in one tensor (one DMA): w4 | win (4 rows) | wout
    # | block-0 xta (4 rows x SB*BC)
    cpk_d = nc.dram_tensor("cpack", [128, 774 + SB * BC], F16,
                           kind="ExternalInput")
    # fp8: identity/16 | block-0 noise (SB*2*BC cols)
    id_d = nc.dram_tensor("ident", [128, 128 + SB * 2 * BC], F8,
                          kind="ExternalInput")
    # state history out: r_d[:, t, c, g*GB+b] = r_{t+1}[neuron c*128+p]
    r_d = nc.dram_tensor("r", [128, T, 2, BC], F16, kind="ExternalOutput")

    with TileContext(nc) as tc:
        with (
            tc.tile_pool(name="consts", bufs=1) as consts,
            tc.tile_pool(name="hist", bufs=2 * G) as histp,
            tc.tile_pool(name="noise", bufs=2) as noisep,
            tc.tile_pool(name="xtap", bufs=2) as xtap,
            tc.tile_pool(name="pv", bufs=3 * G, space="PSUM") as pvp,
            tc.tile_pool(name="scr", bufs=2, space="PSUM") as scrp,
        ):
            cpk_sb = consts.tile_from(cpk_d[:, :])
            idp_sb = consts.tile_from(id_d[:, :])
            id_sb = idp_sb[:, 0:128]
            w4_sb = cpk_sb[:, 0:512]
            win_sb = cpk_sb[:, 512:768]
            wout_sb = cpk_sb[:, 768:774]
            xta0_sb = cpk_sb[0:4, 774:774 + SB * BC].rearrange(
                "p (t b) -> p t b", t=SB)
            noise0_sb = idp_sb[:, 128:128 + SB * 2 * BC].rearrange(
                "p (t c b) -> p t c b", t=SB, c=2)

            # Ordering-only (nosync) chain over every PE matmul: pins the
            # scheduler to the emission order.
            _prev_mm = [None]

            def mm(*args, **kw):
                inst = nc.tensor.matmul(*args, **kw)
                raw = getattr(inst, "ins", inst)
                if _prev_mm[0] is not None:
                    add_dep_helper(raw, _prev_mm[0], sync=False,
                                   reason="pe-stream-order")
                _prev_mm[0] = raw
                return inst

            nxt = None              # prefetched (noise_sb, xta_sb) for chunk+1
            carry_pvs = {}          # cross-chunk prefilled psum tiles
            prev_hists = None
            prev_dma = None         # deferred r-DMA emitter for prev chunk
            for ck, (ts0, TCk) in enumerate(CHUNKS):
                NBLK = TCk // SB
                if nxt is None:
                    noise_sb = noisep.tile([128, TCk, 2 * BC], F8)
                    xta_sb = xtap.tile([4, TCk, BC], F16)
                    nc.sync.dma_start(out=xta_sb[:],
                                      in_=xta_d[:, ts0:ts0 + TCk, :])
                    nc.sync.dma_start(out=noise_sb[:],
                                      in_=noise_d[:, ts0:ts0 + TCk, :])
                else:
                    noise_sb, xta_sb = nxt
                if ck + 1 < len(CHUNKS):
                    nts0, nTC = CHUNKS[ck + 1]
                    n2 = noisep.tile([128, nTC, 2 * BC], F8, name="noise2")
                    nc.sync.dma_start(out=n2[:],
                                      in_=noise_d[:, nts0:nts0 + nTC, :])
                    x2 = xtap.tile([4, nTC, BC], F16, name="xta2")
                    nc.sync.dma_start(out=x2[:],
                                      in_=xta_d[:, nts0:nts0 + nTC, :])
                    nxt = (n2, x2)
                else:
                    nxt = None
                noise_r = noise_sb[:].rearrange("p t (c b) -> p t c b", c=2)
                noise_r2 = (nxt[0][:].rearrange("p t (c b) -> p t c b", c=2)
                            if nxt is not None else None)
                # per-GROUP hist tiles: slot s holds that group's state
                # r_{ts0+s}; slot 0 = carry-in.  Separate tiles per group so
                # a group's chain matmuls wait only on its OWN state update.
                hists = [histp.tile([128, TCk + 1, 2, GB], F16,
                                    name=f"hist{g}") for g in range(G)]
                if ck == 0:
                    for g in range(G):
                        nc.vector.memset(hists[g][:, 0], 0.0)

                pvs = carry_pvs
                carry_pvs = {}

                def emit_drive(key, b0, g, m_c, xt):
                    # PSUM bank protocol: exactly one start=True per bank.
                    gsl = slice(g * GB, (g + 1) * GB)
                    if (key, g) not in pvs:
                        pvs[(key, g)] = pvp.tile([128, SB, 2, GB], F32,
                                                 name="pv", tag="pv")
                    mm(pvs[(key, g)][:, :, m_c, :],
                       win_sb[0:4, m_c * 128:(m_c + 1) * 128],
                       xt[:, b0:b0 + SB, gsl],
                       start=(m_c == 0), stop=False, skip_group_check=True)

                def emit_noise(key, b0, g, s0, s1, nr):
                    gsl = slice(g * GB, (g + 1) * GB)
                    for s in range(s0, s1):
                        mm(pvs[(key, g)][:, s], id_sb[:],
                           nr[:, b0 + s, :, gsl],
                           start=False, stop=False, skip_group_check=True)

                def emit_dummy(b0):
                    # p-state keep-warm: 2x128-col identity matmuls into a
                    # scratch bank (never read).  Dropping PE array duty
                    # demotes the p-state and slows the whole chain.
                    scr = scrp.tile([128, 2, 2 * BC], F32, name="scr")
                    for i in range(2):
                        mm(scr[:], id_sb[:],
                           noise_sb[:, min(b0 + 2 * i, TCk - 2):
                                    min(b0 + 2 * i, TCk - 2) + 2, :],
                           start=(i == 0), stop=(i == 1),
                           skip_group_check=True)

                for blk in range(NBLK):
                    b0 = blk * SB
                    if blk == 0 and (0, 0) not in pvs:
                        # cold start: prefill block 0 from const-packed copies
                        for g in range(G):
                            emit_drive(0, 0, g, 0, xta0_sb)
                            emit_drive(0, 0, g, 1, xta0_sb)
                        for g in range(G):
                            emit_noise(0, 0, g, 0, SB, noise0_sb)
                    # next prefill target: block blk+1, or next chunk's block 0
                    if blk + 1 < NBLK:
                        nkey, nb0, nxta, nnr = blk + 1, b0 + SB, xta_sb, noise_r
                    elif noise_r2 is not None:
                        nkey, nb0, nxta, nnr = "n0", 0, nxt[1], noise_r2
                    else:
                        nkey = None
                    for s in range(SB):
                        l = b0 + s
                        # ---- fillers first (issue during the chain wait,
                        #      ONE unit <= ~130 cols per slot) ----
                        if s == 0 and blk == 0 and prev_dma is not None:
                            prev_dma()      # prev chunk's r-DMAs (off-PE)
                            prev_dma = None
                        if s == 1 and nkey is not None:
                            emit_drive(nkey, nb0, 0, 0, nxta)
                        elif s == 2 and nkey is not None:
                            emit_drive(nkey, nb0, 1, 0, nxta)
                        elif s == 3 and nkey is not None:
                            emit_drive(nkey, nb0, 0, 1, nxta)
                        elif s == 4 and nkey is not None:
                            emit_drive(nkey, nb0, 1, 1, nxta)
                        elif s == 5 and nkey is not None:
                            emit_noise(nkey, nb0, 0, 0, SB, nnr)
                        elif s == 6 and nkey is not None and G > 1:
                            emit_noise(nkey, nb0, 1, 0, SB, nnr)
                        elif s == 7:
                            emit_dummy(b0)
                        # ---- chain ----
                        if l == 0 and ck > 0:
                            rds, rs = prev_hists, prev_TC
                        else:
                            rds, rs = hists, l
                        for g in range(G):
                            pv = pvs[(blk, g)]
                            for k_c in range(2):
                                for m_c in range(2):
                                    mm(pv[:, s, m_c],
                                       w4_sb[:, (2 * k_c + m_c) * 128:
                                             (2 * k_c + m_c + 1) * 128],
                                       rds[g][:, rs, k_c, :],
                                       start=False, stop=(k_c == 1),
                                       skip_group_check=True)
                            # H' = max((1-a)*H, S1)  (single fused DVE op)
                            nc.vector.scalar_tensor_tensor(
                                out=hists[g][:, l + 1, :, :],
                                in0=rds[g][:, rs, :, :],
                                scalar=DECAY,
                                in1=pv[:, s],
                                op0=mybir.AluOpType.mult,
                                op1=mybir.AluOpType.max)
                    if blk > 0:
                        for g in range(G):
                            del pvs[(blk - 1, g)]

                def _emit_r_dma(hists=hists, ts0=ts0, TCk=TCk):
                    for g in range(G):
                        nc.sync.dma_start(
                            out=r_d[:, ts0:ts0 + TCk, :,
                                    g * GB:(g + 1) * GB],
                            in_=hists[g][:, 1:TCk + 1, :, :])
                if nxt is not None:
                    # ship this chunk's r during the next chunk (off-chain;
                    # the DMA waits on this chunk's final state updates)
                    prev_dma = _emit_r_dma
                else:
                    # final chunk: all but the last block early, tail owes
                    # only the last block's slice
                    lb0 = (NBLK - 1) * SB
                    for g in range(G):
                        if lb0 > 0:
                            nc.sync.dma_start(
                                out=r_d[:, ts0:ts0 + lb0, :,
                                        g * GB:(g + 1) * GB],
                                in_=hists[g][:, 1:lb0 + 1, :, :])
                    for g in range(G):
                        nc.sync.dma_start(
                            out=r_d[:, ts0 + lb0:ts0 + TCk, :,
                                    g * GB:(g + 1) * GB],
                            in_=hists[g][:, lb0 + 1:TCk + 1, :, :])
                for g in range(G):
                    if ("n0", g) in pvs:
                        carry_pvs[(0, g)] = pvs.pop(("n0", g))
                prev_hists, prev_TC = hists, TCk
    nc.finalize()
    return nc


def get_nc(T=1000, TC=96, SB=8, G=2):
    key = (T, TC, SB, G)
    if key not in _CACHE:
        _CACHE[key] = _build(T, TC, SB, G)
    return _CACHE[key]


def make_inputs(x, noise, W_in, W_rec, W_out_w, W_out_b, bias):
    """Host-side shard + layout prep.  Returns in_maps for 8 cores.

    Exponential rescaling: the device recurrence uses decay d = fp16(0.8),
    slightly below the true 0.8.  Because relu is positively homogeneous,
    running the recurrence on r~_t = c^t r_t with c = d/0.8 (so 0.8*c = d
    exactly), drive scaled by c^(t+1), and the output rescaled by c^-(t+1)
    on the host reproduces the true-decay dynamics exactly.
    """
    x = np.asarray(x, np.float32)
    noise = np.asarray(noise, np.float32)
    W_in = np.asarray(W_in, np.float32)
    W_rec = np.asarray(W_rec, np.float32)
    W_out_w = np.asarray(W_out_w, np.float32)
    bias = np.asarray(bias, np.float32)
    B, T, _ = x.shape

    cfac = DECAY / (1.0 - ALPHA)                       # 0.99975586
    tfac = np.power(cfac, np.arange(1, T + 1), dtype=np.float64).astype(np.float32)

    cpack = np.zeros((128, 774 + 8 * BC), np.float16)  # w4|win|wout|xta blk0
    wrt = ALPHA * cfac * W_rec.T + DECAY * np.eye(256, dtype=np.float32)
    wrt = wrt.astype(np.float16)                       # [k, m]
    for k_c in range(2):
        for m_c in range(2):
            cpack[:, (2 * k_c + m_c) * 128:(2 * k_c + m_c + 1) * 128] = \
                wrt[128 * k_c:128 * (k_c + 1), 128 * m_c:128 * (m_c + 1)]
    ident = np.zeros((128, 128 + 8 * 2 * BC), F8NP)    # I/16 | noise blk0
    ident[:, 0:128] = (np.eye(128, dtype=np.float32) / NOISE_PREMUL).astype(F8NP)
    cpack[:3, 512:768] = (ALPHA * W_in.T).astype(np.float16)
    cpack[3, 512:768] = (ALPHA * bias).astype(np.float16)
    wt = np.asarray(W_out_w, np.float32).T.astype(np.float16)   # [n, 3]
    for k_c in range(2):
        cpack[:, 768 + 3 * k_c:768 + 3 * (k_c + 1)] = \
            wt[128 * k_c:128 * (k_c + 1)]

    nscale = ALPHA * NOISE_SCALE
    in_maps = []
    for c in range(NCORES):
        b0 = c * BC
        nz = (noise[b0:b0 + BC] * (NOISE_PREMUL * nscale * tfac[None, :, None])
              ).astype(F8NP)                           # [32, T, 256]
        nzt = np.ascontiguousarray(
            nz.reshape(BC, T, 2, 128).transpose(3, 1, 2, 0)).reshape(128, T, 2 * BC)
        xc = x[b0:b0 + BC] * tfac[None, :, None]       # [32, T, 3]
        xta = np.empty((4, T, BC), np.float16)
        xta[:3] = xc.transpose(2, 1, 0).astype(np.float16)
        xta[3] = tfac[:, None]
        cpk = cpack.copy()
        cpk[0:4, 774:774 + 8 * BC] = xta[:, 0:8, :].reshape(4, 8 * BC)
        idp = ident.copy()
        idp[:, 128:128 + 8 * 2 * BC] = nzt[:, 0:8, :].reshape(128, 8 * 2 * BC)
        in_maps.append({
            "noiset": nzt, "xta": xta, "cpack": cpk, "ident": idp,
        })
    return in_maps


def gather_output(results, B, T, W_out_w, W_out_b):
    cfac = DECAY / (1.0 - ALPHA)
    inv = np.power(cfac, -np.arange(1, T + 1), dtype=np.float64).astype(np.float32)
    # r_d[p, t, c, b] = r~_{t+1}[neuron c*128+p] for batch b; project on host
    wt = np.ascontiguousarray(
        np.asarray(W_out_w, np.float32).reshape(3, 2, 128)
        .transpose(2, 1, 0)).reshape(256, 3)           # [(p,c), o]
    out = np.empty((B, T, 3), np.float32)
    for c in range(NCORES):
        r = results[c]["r"]                            # [128, T, 2, BC] f16
        rb = np.ascontiguousarray(r.transpose(3, 1, 0, 2)).reshape(
            BC * T, 256).astype(np.float32)            # [(b,t), (p,c)]
    # wait: wt indexed [(p,c)] must match rb's (p,c) order
        out[c * BC:(c + 1) * BC] = (rb @ wt).reshape(BC, T, 3)
    out *= inv[None, :, None]
    out += np.asarray(W_out_b, np.float32)[None, None, :]
    return out


def kernel(x, noise, W_in, W_rec, W_out_w, W_out_b, bias):
    x = np.asarray(x, np.float32)
    B, T, _ = x.shape
    nc = get_nc(T=T)
    in_maps = make_inputs(x, noise, W_in, W_rec, W_out_w, W_out_b, bias)
    res = run_bass_kernel_spmd(nc, in_maps, list(range(NCORES)))
    return gather_output(res.results, B, T, W_out_w, W_out_b)


# revision 13
# speedup vs baseline: 1.1264x; 1.1264x over previous
"""Trainium2 Bass kernel for the CustomRNN problem (v5).

Model (per batch element b):
    u_t = W_in @ x_t + bias + sigma*sqrt(2*alpha) * noise_t          [N=256]
    r_{t+1} = (1-alpha) * r_t + alpha * relu(W_rec @ r_t + u_t)
    out_t = W_out @ r_{t+1} + b_out                                  [3]

Sharding: data-parallel over batch across 8 cores (32 batch each), weights
replicated.

v5 design notes: the run is latency-bound by the per-step serial cycle
    STT(DVE) -> sem -> 4 chain matmuls -> PSUM pipe -> sem -> STT'
with G=2 staggered 16-batch chains.  In v2..v4 the two groups' state
updates wrote ONE hist tile, so each group's chain matmuls waited on BOTH
groups' DVE ops (coarse region dep + counting semaphore) — putting
STT_A+STT_B (~240ns) on the critical path.  v5 gives each group its OWN
hist tile, so group A's cycle only contains its own STT: predicted cycle
~470ns instead of ~553ns.
Filler work (drive/noise prefill for the next block, y projection of the
previous block) is emitted at the top of designated steps, one filler
TYPE per slot (mixed-stationary filler runs pay ~150ns boundary holes),
each piece <= 128 cols so it never monopolizes the array (the in-order PE
queue turns long fillers into chain stalls).  Keep total PE array duty
comparable to v2 — dropping it demotes the PE p-state and slows the chain
(observed v3: MID->LOW pstate, everything x1.5).
Numerics identical to v2 (fp16 recurrence with exact fp16-decay rescaling,
fp8 x16 noise via identity matmuls, fp32 PSUM).
"""

import numpy as np

import concourse.bacc as bacc
import concourse.mybir as mybir
from concourse.tile import TileContext, add_dep_helper
from concourse.bass_utils import run_bass_kernel_spmd

ALPHA = 0.2
NOISE_SCALE = 0.05 * float(np.sqrt(2 * ALPHA))
DECAY = float(np.float16(1.0 - ALPHA))   # 0.7998046875, exact in fp16
N = 256
NCORES = 8
BC = 32          # batch per core
F16 = mybir.dt.float16
F32 = mybir.dt.float32
F8 = mybir.dt.float8e4      # e4m3
F8NP = mybir.dt.np(mybir.dt.float8e4)
NOISE_PREMUL = 16.0         # fp8 noise stored x16; identity diag = 1/16

_CACHE = {}


def _chunks(T, TC, first):
    """Chunk sizes: a small first chunk (cold-start DMA off the critical
    path), then TC-sized chunks, remainder absorbed at the end."""
    out = []
    o = 0
    if first and T > first:
        out.append((0, first))
        o = first
    while o < T:
        n = min(TC, T - o)
        out.append((o, n))
        o += n
    assert all(n % 8 == 0 for _, n in out)
    return out


def _build(T, TC, SB, G, first=16):
    GB = BC // G
    assert G * GB == BC and SB * 2 * GB * 4 <= 2048 and TC % SB == 0
    CHUNKS = _chunks(T, TC, first)
    nc = bacc.Bacc("TRN2", num_devices=NCORES)

    noise_d = nc.dram_tensor("noiset", [128, T, 2 * BC], F8, kind="ExternalInput")
    xta_d = nc.dram_tensor("xta", [4, T, BC], F16, kind="ExternalInput")
    # all fp16 constants in one tensor (one DMA): w4 | win (4 rows) | wout
    # | block-0 xta (4 rows x SB*BC)
    cpk_d = nc.dram_tensor("cpack", [128, 774 + SB * BC], F16,
                           kind="ExternalInput")
    # fp8: identity/16 | block-0 noise (SB*2*BC cols)
    id_d = nc.dram_tensor("ident", [128, 128 + SB * 2 * BC], F8,
                          kind="ExternalInput")
    # state history out, per-group contiguous (1 DMA descriptor/partition):
    # r_d[g, p, t, c, b] = r~_{t+1}[neuron c*128+p] for batch g*GB+b
    r_d = nc.dram_tensor("r", [G, 128, T, 2, GB], F16, kind="ExternalOutput")

    with TileContext(nc) as tc:
        with (
            tc.tile_pool(name="consts", bufs=1) as consts,
            tc.tile_pool(name="hist", bufs=2 * G) as histp,
            tc.tile_pool(name="noise", bufs=2) as noisep,
            tc.tile_pool(name="xtap", bufs=2) as xtap,
            tc.tile_pool(name="pv", bufs=3 * G, space="PSUM") as pvp,
            tc.tile_pool(name="scr", bufs=2, space="PSUM") as scrp,
        ):
            cpk_sb = consts.tile_from(cpk_d[:, :])
            idp_sb = consts.tile_from(id_d[:, :])
            id_sb = idp_sb[:, 0:128]
            w4_sb = cpk_sb[:, 0:512]
            win_sb = cpk_sb[:, 512:768]
            wout_sb = cpk_sb[:, 768:774]
            xta0_sb = cpk_sb[0:4, 774:774 + SB * BC].rearrange(
                "p (t b) -> p t b", t=SB)
            noise0_sb = idp_sb[:, 128:128 + SB * 2 * BC].rearrange(
                "p (t c b) -> p t c b", t=SB, c=2)

            # Ordering-only (nosync) chain over every PE matmul: pins the
            # scheduler to the emission order.
            _prev_mm = [None]

            def mm(*args, **kw):
                inst = nc.tensor.matmul(*args, **kw)
                raw = getattr(inst, "ins", inst)
                if _prev_mm[0] is not None:
                    add_dep_helper(raw, _prev_mm[0], sync=False,
                                   reason="pe-stream-order")
                _prev_mm[0] = raw
                return inst

            nxt = None              # prefetched (noise_sb, xta_sb) for chunk+1
            carry_pvs = {}          # cross-chunk prefilled psum tiles
            prev_hists = None
            prev_dma = None         # deferred r-DMA emitter for prev chunk
            for ck, (ts0, TCk) in enumerate(CHUNKS):
                NBLK = TCk // SB
                if nxt is None:
                    noise_sb = noisep.tile([128, TCk, 2 * BC], F8)
                    xta_sb = xtap.tile([4, TCk, BC], F16)
                    nc.sync.dma_start(out=xta_sb[:],
                                      in_=xta_d[:, ts0:ts0 + TCk, :])
                    nc.sync.dma_start(out=noise_sb[:],
                                      in_=noise_d[:, ts0:ts0 + TCk, :])
                else:
                    noise_sb, xta_sb = nxt
                if ck + 1 < len(CHUNKS):
                    nts0, nTC = CHUNKS[ck + 1]
                    n2 = noisep.tile([128, nTC, 2 * BC], F8, name="noise2")
                    nc.sync.dma_start(out=n2[:],
                                      in_=noise_d[:, nts0:nts0 + nTC, :])
                    x2 = xtap.tile([4, nTC, BC], F16, name="xta2")
                    nc.sync.dma_start(out=x2[:],
                                      in_=xta_d[:, nts0:nts0 + nTC, :])
                    nxt = (n2, x2)
                else:
                    nxt = None
                noise_r = noise_sb[:].rearrange("p t (c b) -> p t c b", c=2)
                noise_r2 = (nxt[0][:].rearrange("p t (c b) -> p t c b", c=2)
                            if nxt is not None else None)
                # per-GROUP hist tiles: slot s holds that group's state
                # r_{ts0+s}; slot 0 = carry-in.  Separate tiles per group so
                # a group's chain matmuls wait only on its OWN state update.
                hists = [histp.tile([128, TCk + 1, 2, GB], F16,
                                    name=f"hist{g}") for g in range(G)]
                if ck == 0:
                    for g in range(G):
                        nc.vector.memset(hists[g][:, 0], 0.0)

                pvs = carry_pvs
                carry_pvs = {}

                def emit_drive(key, b0, g, m_c, xt):
                    # PSUM bank protocol: exactly one start=True per bank.
                    gsl = slice(g * GB, (g + 1) * GB)
                    if (key, g) not in pvs:
                        pvs[(key, g)] = pvp.tile([128, SB, 2, GB], F32,
                                                 name="pv", tag="pv")
                    mm(pvs[(key, g)][:, :, m_c, :],
                       win_sb[0:4, m_c * 128:(m_c + 1) * 128],
                       xt[:, b0:b0 + SB, gsl],
                       start=(m_c == 0), stop=False, skip_group_check=True)

                def emit_noise(key, b0, g, s0, s1, nr):
                    gsl = slice(g * GB, (g + 1) * GB)
                    for s in range(s0, s1):
                        mm(pvs[(key, g)][:, s], id_sb[:],
                           nr[:, b0 + s, :, gsl],
                           start=False, stop=False, skip_group_check=True)

                def emit_dummy(b0):
                    # p-state keep-warm: 2x128-col identity matmuls into a
                    # scratch bank (never read).  Dropping PE array duty
                    # demotes the p-state and slows the whole chain.
                    scr = scrp.tile([128, 2, 2 * BC], F32, name="scr")
                    for i in range(2):
                        mm(scr[:], id_sb[:],
                           noise_sb[:, min(b0 + 2 * i, TCk - 2):
                                    min(b0 + 2 * i, TCk - 2) + 2, :],
                           start=(i == 0), stop=(i == 1),
                           skip_group_check=True)

                for blk in range(NBLK):
                    b0 = blk * SB
                    if blk == 0 and (0, 0) not in pvs:
                        # cold start: prefill block 0 from const-packed copies
                        for g in range(G):
                            emit_drive(0, 0, g, 0, xta0_sb)
                            emit_drive(0, 0, g, 1, xta0_sb)
                        for g in range(G):
                            emit_noise(0, 0, g, 0, SB, noise0_sb)
                    # next prefill target: block blk+1, or next chunk's block 0
                    if blk + 1 < NBLK:
                        nkey, nb0, nxta, nnr = blk + 1, b0 + SB, xta_sb, noise_r
                    elif noise_r2 is not None:
                        nkey, nb0, nxta, nnr = "n0", 0, nxt[1], noise_r2
                    else:
                        nkey = None
                    for s in range(SB):
                        l = b0 + s
                        # ---- fillers first (issue during the chain wait,
                        #      ONE unit <= ~130 cols per slot) ----
                        if s == 0 and blk == 0 and prev_dma is not None:
                            prev_dma()      # prev chunk's r-DMAs (off-PE)
                            prev_dma = None
                        if s == 1 and nkey is not None:
                            emit_drive(nkey, nb0, 0, 0, nxta)
                        elif s == 2 and nkey is not None:
                            emit_drive(nkey, nb0, 1, 0, nxta)
                        elif s == 3 and nkey is not None:
                            emit_drive(nkey, nb0, 0, 1, nxta)
                        elif s == 4 and nkey is not None:
                            emit_drive(nkey, nb0, 1, 1, nxta)
                        elif s == 5 and nkey is not None:
                            emit_noise(nkey, nb0, 0, 0, SB, nnr)
                        elif s == 6 and nkey is not None and G > 1:
                            emit_noise(nkey, nb0, 1, 0, SB, nnr)
                        elif s == 7:
                            emit_dummy(b0)
                        # ---- chain ----
                        if l == 0 and ck > 0:
                            rds, rs = prev_hists, prev_TC
                        else:
                            rds, rs = hists, l
                        for g in range(G):
                            pv = pvs[(blk, g)]
                            for k_c in range(2):
                                for m_c in range(2):
                                    mm(pv[:, s, m_c],
                                       w4_sb[:, (2 * k_c + m_c) * 128:
                                             (2 * k_c + m_c + 1) * 128],
                                       rds[g][:, rs, k_c, :],
                                       start=False, stop=(k_c == 1),
                                       skip_group_check=True)
                            # H' = max((1-a)*H, S1)  (single fused DVE op)
                            nc.vector.scalar_tensor_tensor(
                                out=hists[g][:, l + 1, :, :],
                                in0=rds[g][:, rs, :, :],
                                scalar=DECAY,
                                in1=pv[:, s],
                                op0=mybir.AluOpType.mult,
                                op1=mybir.AluOpType.max)
                    if blk > 0:
                        for g in range(G):
                            del pvs[(blk - 1, g)]

                def _emit_r_dma(hists=hists, ts0=ts0, TCk=TCk):
                    for g in range(G):
                        nc.sync.dma_start(
                            out=r_d[g, :, ts0:ts0 + TCk, :, :],
                            in_=hists[g][:, 1:TCk + 1, :, :])
                if nxt is not None:
                    # ship this chunk's r during the next chunk (off-chain;
                    # the DMA waits on this chunk's final state updates)
                    prev_dma = _emit_r_dma
                else:
                    # final chunk: all but the last block early, tail owes
                    # only the last block's slice
                    lb0 = (NBLK - 1) * SB
                    for g in range(G):
                        if lb0 > 0:
                            nc.sync.dma_start(
                                out=r_d[g, :, ts0:ts0 + lb0, :, :],
                                in_=hists[g][:, 1:lb0 + 1, :, :])
                    for g in range(G):
                        nc.sync.dma_start(
                            out=r_d[g, :, ts0 + lb0:ts0 + TCk, :, :],
                            in_=hists[g][:, lb0 + 1:TCk + 1, :, :])
                for g in range(G):
                    if ("n0", g) in pvs:
                        carry_pvs[(0, g)] = pvs.pop(("n0", g))
                prev_hists, prev_TC = hists, TCk
    nc.finalize()
    return nc


def get_nc(T=1000, TC=96, SB=8, G=2):
    key = (T, TC, SB, G)
    if key not in _CACHE:
        _CACHE[key] = _build(T, TC, SB, G)
    return _CACHE[key]


def make_inputs(x, noise, W_in, W_rec, W_out_w, W_out_b, bias):
    """Host-side shard + layout prep.  Returns in_maps for 8 cores.

    Exponential rescaling: the device recurrence uses decay d = fp16(0.8),
    slightly below the true 0.8.  Because relu is positively homogeneous,
    running the recurrence on r~_t = c^t r_t with c = d/0.8 (so 0.8*c = d
    exactly), drive scaled by c^(t+1), and the output rescaled by c^-(t+1)
    on the host reproduces the true-decay dynamics exactly.
    """
    x = np.asarray(x, np.float32)
    noise = np.asarray(noise, np.float32)
    W_in = np.asarray(W_in, np.float32)
    W_rec = np.asarray(W_rec, np.float32)
    W_out_w = np.asarray(W_out_w, np.float32)
    bias = np.asarray(bias, np.float32)
    B, T, _ = x.shape

    cfac = DECAY / (1.0 - ALPHA)                       # 0.99975586
    tfac = np.power(cfac, np.arange(1, T + 1), dtype=np.float64).astype(np.float32)

    cpack = np.zeros((128, 774 + 8 * BC), np.float16)  # w4|win|wout|xta blk0
    wrt = ALPHA * cfac * W_rec.T + DECAY * np.eye(256, dtype=np.float32)
    wrt = wrt.astype(np.float16)                       # [k, m]
    for k_c in range(2):
        for m_c in range(2):
            cpack[:, (2 * k_c + m_c) * 128:(2 * k_c + m_c + 1) * 128] = \
                wrt[128 * k_c:128 * (k_c + 1), 128 * m_c:128 * (m_c + 1)]
    ident = np.zeros((128, 128 + 8 * 2 * BC), F8NP)    # I/16 | noise blk0
    ident[:, 0:128] = (np.eye(128, dtype=np.float32) / NOISE_PREMUL).astype(F8NP)
    cpack[:3, 512:768] = (ALPHA * W_in.T).astype(np.float16)
    cpack[3, 512:768] = (ALPHA * bias).astype(np.float16)
    wt = np.asarray(W_out_w, np.float32).T.astype(np.float16)   # [n, 3]
    for k_c in range(2):
        cpack[:, 768 + 3 * k_c:768 + 3 * (k_c + 1)] = \
            wt[128 * k_c:128 * (k_c + 1)]

    nscale = ALPHA * NOISE_SCALE
    in_maps = []
    for c in range(NCORES):
        b0 = c * BC
        nz = (noise[b0:b0 + BC] * (NOISE_PREMUL * nscale * tfac[None, :, None])
              ).astype(F8NP)                           # [32, T, 256]
        nzt = np.ascontiguousarray(
            nz.reshape(BC, T, 2, 128).transpose(3, 1, 2, 0)).reshape(128, T, 2 * BC)
        xc = x[b0:b0 + BC] * tfac[None, :, None]       # [32, T, 3]
        xta = np.empty((4, T, BC), np.float16)
        xta[:3] = xc.transpose(2, 1, 0).astype(np.float16)
        xta[3] = tfac[:, None]
        cpk = cpack.copy()
        cpk[0:4, 774:774 + 8 * BC] = xta[:, 0:8, :].reshape(4, 8 * BC)
        idp = ident.copy()
        idp[:, 128:128 + 8 * 2 * BC] = nzt[:, 0:8, :].reshape(128, 8 * 2 * BC)
        in_maps.append({
            "noiset": nzt, "xta": xta, "cpack": cpk, "ident": idp,
        })
    return in_maps


def gather_output(results, B, T, W_out_w, W_out_b):
    cfac = DECAY / (1.0 - ALPHA)
    inv = np.power(cfac, -np.arange(1, T + 1), dtype=np.float64).astype(np.float32)
    # r_d[p, t, c, b] = r~_{t+1}[neuron c*128+p] for batch b; project on host
    wt = np.ascontiguousarray(
        np.asarray(W_out_w, np.float32).reshape(3, 2, 128)
        .transpose(2, 1, 0)).reshape(256, 3)           # [(p,c), o]
    out = np.empty((B, T, 3), np.float32)
    for c in range(NCORES):
        r = results[c]["r"]                      # [G, 128, T, 2, GB] f16
        rb = np.ascontiguousarray(r.transpose(0, 4, 2, 1, 3)).reshape(
            BC * T, 256).astype(np.float32)      # [(g,b,t), (p,c)]
        out[c * BC:(c + 1) * BC] = (rb @ wt).reshape(BC, T, 3)
    out *= inv[None, :, None]
    out += np.asarray(W_out_b, np.float32)[None, None, :]
    return out


def kernel(x, noise, W_in, W_rec, W_out_w, W_out_b, bias):
    x = np.asarray(x, np.float32)
    B, T, _ = x.shape
    nc = get_nc(T=T)
    in_maps = make_inputs(x, noise, W_in, W_rec, W_out_w, W_out_b, bias)
    res = run_bass_kernel_spmd(nc, in_maps, list(range(NCORES)))
    return gather_output(res.results, B, T, W_out_w, W_out_b)
